# revision 35
# baseline (speedup 1.0000x reference)
"""Trainium2 Bass kernel for nn_DyResGEN (3-block GENConv GNN with top-k pooling).

Strategy (8 NeuronCores, SPMD):
  * Edges are partitioned across cores by destination-node ranges, so the
    per-destination segment softmax/sum is core-local.  The full projected
    node table ("htab" = x @ Wsrc) is replicated per core; rows are fetched
    with indirect-DMA gathers (h[src]).
  * Per 128-edge chunk: messages m = relu(h[src] + ea0*w0 + ea1*w1) + eps and
    ex = exp(t*m) on DVE/ACT; a one-hot selection matrix S (dst-local ids vs
    iota) reduces [ex | m*ex] into per-destination-tile PSUM accumulators on
    the PE (matmul contracts the edge dim).  Softmax max-subtraction is
    dropped: logits are O(10) so exp is safe in fp32, and alpha is invariant.
  * agg = num/den with reciprocal via exp(-ln(x)); GENConv MLP per 128-node
    tile; batched LayerNorm (+relu) across the core's node slice; each launch
    also computes the next launch's projected node table slice.
  * Host does index work only: edge filtering/relabeling, top-k, packing,
    slicing/concatenation between the 6 launches (conv0+skip per block).
"""

import math
import os
import sys

import ml_dtypes
import numpy as np

try:
    import concourse  # noqa: F401
except ImportError:  # pragma: no cover
    sys.path.insert(0, "/opt/trn_rl_repo")

import concourse.bacc as bacc
import concourse.bass as bass
import concourse.mybir as mybir
import concourse.tile as tile
from concourse.bass_utils import run_bass_kernel_spmd

NC = 8           # NeuronCores
G = 8            # 128-edge chunks per batch
MSG_EPS = 1e-7
POOL_RATIO = 0.5

F32 = mybir.dt.float32
BF16 = mybir.dt.bfloat16
I32 = mybir.dt.int32
AF = mybir.ActivationFunctionType
OP = mybir.AluOpType
AX = mybir.AxisListType

LAUNCH_STATS = []  # filled per launch; read by test.py
MM_DT = F32 if os.environ.get("KMM_F32") else BF16


def _ceil(a, b):
    return -(-a // b)


# --------------------------------------------------------------------------
# host-side graph packing
# --------------------------------------------------------------------------

def pack_block_edges(src, dst, ea, n, vals=None):
    """Pack a block's edges into per-core chunked format with a schedule
    shared across cores (required for SPMD).

    Chunks are 128 edges; within a destination tile, edges whose source is
    < 32768 ("lo") come first, then "hi" sources (dma_gather idx is int16).
    Returns meta (shared schedule + per-batch gather runs) and per-core
    arrays EP [NB,128,4G] f32 (dstl|ea0|ea1|val) and IDX [NB,128,8G] i16
    (wrapped per gather run).
    """
    HI0 = 32768
    Sb = _ceil(n, NC)
    T = _ceil(Sb, 128)
    npad = (NC - 1) * Sb + T * 128
    has_hi = npad > HI0

    core = dst // Sb
    local = dst - core * Sb
    tl = local // 128
    dl = (local - tl * 128).astype(np.float32)
    hi = (src >= HI0).astype(np.int64)

    counts = np.zeros((NC, T, 2), np.int64)
    np.add.at(counts, (core, tl, hi), 1)
    K2 = -(-counts.max(axis=0) // 128)          # [T, 2] chunks per (tile, class)
    K2[:, 0] = np.maximum(K2[:, 0], (K2.sum(1) == 0).astype(np.int64))
    Ktot = int(K2.sum())
    NB = _ceil(Ktot, G)
    K2[T - 1, 0] += NB * G - Ktot               # tail pads: lo class, last tile

    # chunk stream: per tile lo chunks then hi chunks; flags per chunk
    chunk_tile = []
    chunk_hi = []
    sched = []
    for t in range(T):
        tot = int(K2[t].sum())
        k = 0
        for cls in (0, 1):
            for _ in range(int(K2[t, cls])):
                chunk_tile.append(t)
                chunk_hi.append(cls)
                sched.append((t, k == 0, k == tot - 1))
                k += 1
    assert len(sched) == NB * G
    chunk_hi = np.array(chunk_hi)

    # per-batch gather runs: maximal same-class chunk spans
    runs = []
    for b in range(NB):
        rb = []
        g = 0
        while g < G:
            g1 = g + 1
            while g1 < G and chunk_hi[b * G + g1] == chunk_hi[b * G + g]:
                g1 += 1
            rb.append((g, g1, int(chunk_hi[b * G + g])))
            g = g1
        runs.append(tuple(rb))

    # chunk slot offsets per (tile, class)
    slot0 = np.zeros((T, 2), np.int64)
    acc = 0
    for t in range(T):
        slot0[t, 0] = acc
        acc += int(K2[t, 0])
        slot0[t, 1] = acc
        acc += int(K2[t, 1])

    order = np.lexsort((hi, tl, core))
    src_s, tl_s, core_s, dl_s, hi_s = (src[order], tl[order], core[order],
                                       dl[order], hi[order])
    ea_s = ea[order]
    val_s = (vals[src[order]] if vals is not None
             else np.zeros(len(order), np.float32))

    n_chunks = NB * G
    per_core = []
    for c in range(NC):
        dstl_c = np.full((n_chunks, 128), 255.0, np.float32)
        ea0_c = np.zeros((n_chunks, 128), np.float32)
        ea1_c = np.zeros((n_chunks, 128), np.float32)
        val_c = np.zeros((n_chunks, 128), np.float32)
        idx_c = np.zeros((n_chunks, 128), np.int32)
        sel = np.where(core_s == c)[0]
        if len(sel):
            key = tl_s[sel] * 2 + hi_s[sel]
            bounds = np.searchsorted(key, np.arange(2 * T + 1))
            for t in range(T):
                for cls in (0, 1):
                    a, b = bounds[2 * t + cls], bounds[2 * t + cls + 1]
                    cnt = b - a
                    if cnt == 0:
                        continue
                    q0 = int(slot0[t, cls])
                    nch = _ceil(cnt, 128)
                    pad = nch * 128 - cnt
                    ii = sel[a:b]
                    fdl = np.concatenate(
                        [dl_s[ii], np.full(pad, 255.0, np.float32)])
                    fsr = np.concatenate(
                        [(src_s[ii] - cls * HI0).astype(np.int32),
                         np.zeros(pad, np.int32)])
                    fea = np.concatenate(
                        [ea_s[ii], np.zeros((pad, 2), np.float32)])
                    fva = np.concatenate(
                        [val_s[ii], np.zeros(pad, np.float32)])
                    dstl_c[q0:q0 + nch] = fdl.reshape(nch, 128)
                    idx_c[q0:q0 + nch] = fsr.reshape(nch, 128)
                    ea0_c[q0:q0 + nch] = fea[:, 0].reshape(nch, 128)
                    ea1_c[q0:q0 + nch] = fea[:, 1].reshape(nch, 128)
                    val_c[q0:q0 + nch] = fva.reshape(nch, 128)
        dst4 = dstl_c.reshape(NB, G, 128).transpose(0, 2, 1)
        e04 = ea0_c.reshape(NB, G, 128).transpose(0, 2, 1)
        e14 = ea1_c.reshape(NB, G, 128).transpose(0, 2, 1)
        v4 = val_c.reshape(NB, G, 128).transpose(0, 2, 1)
        EP = np.ascontiguousarray(
            np.concatenate([dst4, e04, e14, v4], axis=2), np.float32)
        # wrapped int16 idx layout, per gather run
        IDX16 = np.zeros((NB, 128, 8 * G), np.int16)
        for b in range(NB):
            for (g0, g1, cls) in runs[b]:
                ln = g1 - g0
                flat = idx_c[b * G + g0: b * G + g1].reshape(ln * 128)
                wrap = flat.reshape(8 * ln, 16).T.astype(np.int16)  # [16, 8ln]
                IDX16[b, :, 8 * g0:8 * g1] = np.tile(wrap, (8, 1))
        EAT = np.stack([ea0_c.reshape(NB, G * 128),
                        ea1_c.reshape(NB, G * 128)], axis=1)
        EB = np.concatenate([EP.view(np.uint8).reshape(NB, 128, 16 * G),
                             IDX16.view(np.uint8).reshape(NB, 128, 16 * G)],
                            axis=2)
        per_core.append({"EB": np.ascontiguousarray(EB),
                         "EAT": np.ascontiguousarray(
                             EAT.astype(ml_dtypes.bfloat16))})

    meta = {"n": n, "Sb": Sb, "T": T, "NB": NB, "npad": npad,
            "sched": tuple(sched), "runs": tuple(runs), "has_hi": has_hi}
    return meta, per_core


# --------------------------------------------------------------------------
# device program builder
# --------------------------------------------------------------------------

_PROGRAM_CACHE = {}


def build_conv_program(cfg):
    key = (cfg["h"], cfg["helem"], cfg["hn"], cfg["T"], cfg["NB"],
           cfg["npad"], cfg["gated"], cfg["is_skip"], cfg["has_scores"],
           cfg["t_scalar"], cfg["wnorm"], cfg["sched"], cfg["runs"])
    if key in _PROGRAM_CACHE:
        return _PROGRAM_CACHE[key]

    h = cfg["h"]
    helem = cfg["helem"]
    hn = cfg["hn"]
    T = cfg["T"]
    NB = cfg["NB"]
    npad = cfg["npad"]
    sched = cfg["sched"]
    gated = cfg["gated"]
    is_skip = cfg["is_skip"]
    has_scores = cfg["has_scores"]
    t_scalar = cfg["t_scalar"]
    wnorm = cfg["wnorm"]
    runs = cfg["runs"]
    has_hi = cfg["has_hi"]
    hpad = _ceil(h, 64) * 64
    HI0 = 32768
    nlo = min(npad, HI0)
    nhi = npad - nlo
    h2 = 2 * h
    nk2 = _ceil(h2, 128)

    nc = bacc.Bacc("TRN2", num_devices=NC,
                   dynamic_dma_scratch_size=65536)

    htabL = nc.dram_tensor("htabL", [nlo, hpad], F32, kind="ExternalInput")
    if has_hi:
        htabH = nc.dram_tensor("htabH", [nhi, hpad], F32,
                               kind="ExternalInput")
    hownd = nc.dram_tensor("hown", [128, T * helem], F32, kind="ExternalInput")
    EBd = nc.dram_tensor("EB", [NB, 128, 32 * G], mybir.dt.uint8,
                         kind="ExternalInput")
    EATd = nc.dram_tensor("EAT", [NB, 2, G * 128], BF16, kind="ExternalInput")
    wedged = nc.dram_tensor("wedge", [2, h], BF16, kind="ExternalInput")
    W1d = nc.dram_tensor("W1", [h, h2], F32, kind="ExternalInput")
    W2d = nc.dram_tensor("W2", [h2, h], F32, kind="ExternalInput")
    iotad = nc.dram_tensor("iota", [128, 128], F32, kind="ExternalInput")
    identd = nc.dram_tensor("ident", [128, 128], F32, kind="ExternalInput")
    bconstd = nc.dram_tensor("bconst", [128, 4], F32, kind="ExternalInput")
    if hn:
        Wnd = nc.dram_tensor("Wn", [h, hn], F32, kind="ExternalInput")
    if is_skip:
        xresd = nc.dram_tensor("xres", [128, T * h], F32, kind="ExternalInput")
    if has_scores:
        poolwd = nc.dram_tensor("poolw", [128, h], F32, kind="ExternalInput")
        SCd = nc.dram_tensor("SC", [128, T], F32, kind="ExternalOutput")
    XOUTd = nc.dram_tensor("XOUT", [128, T * h], F32, kind="ExternalOutput")
    import os as _os
    if _os.environ.get("KDBG"):
        DBGXd = nc.dram_tensor("DBGX", [128, G * h2], BF16, kind="ExternalOutput")
        DBGSd = nc.dram_tensor("DBGS", [128, G * 128], BF16, kind="ExternalOutput")
        DBGGd = nc.dram_tensor("DBGG", [128, G * hpad], F32, kind="ExternalOutput")
    if hn:
        HNd = nc.dram_tensor("HN", [128, T * hn], F32, kind="ExternalOutput")

    with tile.TileContext(nc) as tc:
        with (
            tc.tile_pool(name="const", bufs=1) as constp,
            tc.tile_pool(name="edgein", bufs=4) as edgein,
            tc.tile_pool(name="gat", bufs=4) as gatherp,
            tc.tile_pool(name="msg", bufs=3) as msgp,
            tc.tile_pool(name="exw", bufs=3) as exwp,
            tc.tile_pool(name="sel", bufs=3) as selp,
            tc.tile_pool(name="node", bufs=2) as nodep,
            tc.tile_pool(name="big", bufs=1) as bigp,
            tc.tile_pool(name="pse", bufs=2, space="PSUM") as psum_e,
            tc.tile_pool(name="psb", bufs=3, space="PSUM") as psum_b,
            tc.tile_pool(name="psn", bufs=2, space="PSUM") as psum_n,
            tc.tile_pool(name="pst", bufs=1, space="PSUM") as psum_t,
        ):
            from concourse import library_config
            nc.gpsimd.load_library(library_config.mlp)
            # ---------------- constants ----------------
            iota_t = constp.tile([128, 128], F32, tag="iota")
            nc.sync.dma_start(iota_t[:], iotad[:, :])
            iotab_t = constp.tile([128, 128], BF16, tag="iotab")
            nc.vector.tensor_copy(iotab_t[:], iota_t[:])
            ident_t = constp.tile([128, 128], F32, tag="ident")
            nc.sync.dma_start(ident_t[:], identd[:, :])
            bc_t = constp.tile([128, 4], F32, tag="bconst")
            nc.sync.dma_start(bc_t[:], bconstd[:, :])
            wedge_t = constp.tile([128, h], BF16, tag="wedge")
            nc.sync.dma_start(wedge_t[:2, :], wedged[:, :])
            W1_t = constp.tile([128, h2], F32, tag="W1")
            nc.sync.dma_start(W1_t[:h, :], W1d[:, :])
            W2_t = constp.tile([128, nk2 * h], F32, tag="W2")
            for k in range(nk2):
                kk = min(128, h2 - k * 128)
                nc.sync.dma_start(W2_t[:kk, k * h:(k + 1) * h],
                                  W2d[k * 128:k * 128 + kk, :])
            if hn:
                Wn_t = constp.tile([128, hn], F32, tag="Wn")
                nc.sync.dma_start(Wn_t[:h, :], Wnd[:, :])
            if has_scores:
                poolw_t = constp.tile([128, h], F32, tag="poolw")
                nc.sync.dma_start(poolw_t[:], poolwd[:, :])

            hown_all = bigp.tile([128, T * helem], F32, tag="hownall")
            nc.sync.dma_start(hown_all[:], hownd[:, :])
            if is_skip:
                xres_all = bigp.tile([128, T * h], F32, tag="xresall")
                nc.sync.dma_start(xres_all[:], xresd[:, :])
            xnew_all = bigp.tile([128, T * h], F32, tag="xnew")
            xc_all = bigp.tile([128, T * h], F32, tag="xc")
            hpre_all = bigp.tile([128, T * h], F32, tag="hpre")
            if hn:
                hn_all = bigp.tile([128, T * hn], F32, tag="hnall")
            if has_scores:
                sc_all = bigp.tile([128, T], F32, tag="scall")
            stat_s = bigp.tile([128, T], F32, tag="stats")
            stat_m = bigp.tile([128, T], F32, tag="statm")
            stat_v = bigp.tile([128, T], F32, tag="statv")
            stat_r = bigp.tile([128, T], F32, tag="statr")
            sq_all = bigp.tile([128, T * h], F32, tag="sqall")

            psum_of = {}

            def node_phase(t, ps):
                # agg = num/den via DVE reciprocal (avoids ACT Ln/Exp table
                # thrash against the edge-phase Exp)
                d1 = nodep.tile([128, h], F32, tag="lnd")
                nc.vector.tensor_scalar_add(d1[:], ps[:, 0:h], 1e-16)
                rden = nodep.tile([128, h], F32, tag="rden")
                nc.vector.reciprocal(rden[:], d1[:])
                agg = nodep.tile([128, h], F32, tag="agg")
                nc.vector.tensor_tensor(agg[:], ps[:, h:h2], rden[:], OP.mult)
                hown = hown_all[:, t * helem:(t + 1) * helem]
                o_t = nodep.tile([128, h], F32, tag="o")
                if gated:
                    nc.vector.scalar_tensor_tensor(
                        o_t[:], hown[:, 0:h], hown[:, h:h + 1], agg[:],
                        OP.mult, OP.add)
                else:
                    nc.vector.tensor_tensor(o_t[:], hown[:, 0:h], agg[:],
                                            OP.add)
                # MLP
                tp = psum_t.tile([128, 128], F32, tag="tp")
                nc.tensor.transpose(tp[:h, :], o_t[:], ident_t[:])
                oT = nodep.tile([128, 128], F32, tag="oT")
                nc.scalar.copy(oT[:h, :], tp[:h, :])
                y1p = psum_n.tile([128, h2], F32, tag="mlp")
                nc.tensor.matmul(y1p[:], oT[:h, :], W1_t[:h, :])
                y1r = nodep.tile([128, h2], F32, tag="y1r")
                nc.scalar.activation(y1r[:], y1p[:], AF.Relu)
                yp = psum_n.tile([128, h], F32, tag="mlp", name="yp")
                for k in range(nk2):
                    kk = min(128, h2 - k * 128)
                    tp2 = psum_t.tile([128, 128], F32, tag="tp")
                    nc.tensor.transpose(tp2[:kk, :],
                                        y1r[:, k * 128:k * 128 + kk],
                                        ident_t[:])
                    y1rT = nodep.tile([128, 128], F32, tag="y1rT")
                    nc.scalar.copy(y1rT[:kk, :], tp2[:kk, :])
                    nc.tensor.matmul(yp[:], y1rT[:kk, :],
                                     W2_t[:kk, k * h:(k + 1) * h],
                                     start=(k == 0), stop=(k == nk2 - 1))
                xn = xnew_all[:, t * h:(t + 1) * h]
                if is_skip:
                    nc.vector.tensor_tensor(
                        xn, yp[:], xres_all[:, t * h:(t + 1) * h], OP.add)
                else:
                    nc.scalar.copy(xn, yp[:])

            # ---------------- edge phase ----------------
            for b in range(NB):
                eb_t = edgein.tile([128, 32 * G], mybir.dt.uint8, tag="eb")
                nc.sync.dma_start(eb_t[:], EBd[b, :, :])
                ep_t = eb_t[:, 0:16 * G].bitcast(F32)
                ix_t = eb_t[:, 16 * G:32 * G].bitcast(mybir.dt.int16)
                gt = gatherp.tile([128, G * hpad], F32, tag="gt")
                gt3 = gt[:].rearrange("p (g e) -> p g e", g=G)
                for (g0, g1, cls) in runs[b]:
                    ln = g1 - g0
                    tab = htabH if cls else htabL
                    nc.gpsimd.dma_gather(
                        gt3[:, g0:g1, :], tab[:, :],
                        ix_t[:, 8 * g0:8 * g1], ln * 128, ln * 128, hpad)
                # eemb = ea^T @ Wedge on the PE (per-chunk stationary)
                eat_t = edgein.tile([128, G * 128], BF16, tag="eat")
                nc.sync.dma_start(eat_t[:2, :], EATd[b, :, :])
                NH = 1 if G * h * 4 <= 2048 else 2
                G2 = G // NH
                emb_halves = []
                for hh in range(NH):
                    emb_ps = psum_b.tile([128, G2 * h], F32, tag="emb",
                                         name="emb")
                    for gg in range(G2):
                        g = hh * G2 + gg
                        nc.tensor.matmul(emb_ps[:, gg * h:(gg + 1) * h],
                                         eat_t[:2, g * 128:(g + 1) * 128],
                                         wedge_t[:2, :], start=True,
                                         stop=True)
                    emb_halves.append(emb_ps)
                d_t = msgp.tile([128, G * h], F32, tag="d")
                d3 = d_t[:].rearrange("p (g h) -> p g h", g=G)
                if gated:
                    for g in range(G):
                        emb_ps = emb_halves[g // G2]
                        gg = g % G2
                        nc.vector.scalar_tensor_tensor(
                            d_t[:, g * h:(g + 1) * h], gt3[:, g, 0:h],
                            ep_t[:, 3 * G + g:3 * G + g + 1],
                            emb_ps[:, gg * h:(gg + 1) * h],
                            OP.mult, OP.add)
                else:
                    for hh in range(NH):
                        nc.vector.tensor_tensor(
                            d3[:, hh * G2:(hh + 1) * G2, :],
                            emb_halves[hh][:].rearrange(
                                "p (g h) -> p g h", g=G2),
                            gt3[:, hh * G2:(hh + 1) * G2, 0:h], OP.add)
                # me = relu(d) + eps on DVE (tensor_scalar, two scalars)
                m_t = msgp.tile([128, G * h], BF16, tag="m")
                nc.vector.tensor_scalar(m_t[:], d_t[:], 0.0, MSG_EPS,
                                        OP.max, OP.add)
                exw = exwp.tile([128, G * h2], MM_DT, tag="exw")
                exv = exw[:].rearrange("p (g h) -> p g h", g=G)
                mv = m_t[:].rearrange("p (g h) -> p g h", g=G)
                nc.scalar.activation(exv[:, :, 0:h], mv, AF.Exp,
                                     scale=t_scalar)
                nc.vector.tensor_tensor(exv[:, :, h:h2], mv,
                                        exv[:, :, 0:h], OP.mult)
                # S one-hot: per-chunk tensor_scalar vs bf16 iota (4x mode)
                S_t = selp.tile([128, G * 128], MM_DT, tag="S")
                Sv = S_t[:].rearrange("p (g s) -> p g s", g=G)
                for g in range(G):
                    nc.vector.tensor_scalar(
                        S_t[:, g * 128:(g + 1) * 128], iotab_t[:],
                        ep_t[:, g:g + 1], None, OP.is_equal)
                if _os.environ.get("KDBG") and b == 0:
                    nc.sync.dma_start(DBGXd[:, :], exw[:])
                    nc.sync.dma_start(DBGSd[:, :], S_t[:])
                    nc.sync.dma_start(DBGGd[:, :], gt[:])
                for g in range(G):
                    t_id, st, sp = sched[b * G + g]
                    if st:
                        psum_of[t_id] = psum_e.tile([128, h2], F32, tag="eps", name="eps")
                    ps = psum_of[t_id]
                    nc.tensor.matmul(ps[:], Sv[:, g, :], exv[:, g, :],
                                     start=st, stop=sp)
                    if sp:
                        node_phase(t_id, ps)

            # ---- batched LayerNorm (+relu), scores, next-table projection,
            # emitted per tile-group with disjoint AP ranges so the scheduler
            # can overlap finalization with the remaining edge phase ----
            def finalize(t0, t1):
                Tg = t1 - t0
                fs = slice(t0 * h, t1 * h)
                ts_ = slice(t0, t1)
                xa3 = (xnew_all[:, fs].rearrange("p (t h) -> p t h", t=Tg))
                nc.vector.tensor_reduce(stat_s[:, ts_], xa3, AX.X, OP.add)
                nc.scalar.activation(stat_m[:, ts_], stat_s[:, ts_], AF.Copy,
                                     scale=-1.0 / h)
                mue = (stat_m[:, ts_].unsqueeze(2)
                       .broadcast_to([128, Tg, h]))
                xc3 = xc_all[:, fs].rearrange("p (t h) -> p t h", t=Tg)
                nc.vector.tensor_tensor(xc3, xa3, mue, OP.add)
                nc.scalar.activation(sq_all[:, fs], xc_all[:, fs], AF.Square)
                sq3 = sq_all[:, fs].rearrange("p (t h) -> p t h", t=Tg)
                nc.vector.tensor_reduce(stat_v[:, ts_], sq3, AX.X, OP.add)
                nc.scalar.activation(stat_v[:, ts_], stat_v[:, ts_], AF.Copy,
                                     scale=1.0 / h, bias=1e-5)
                nc.vector.reciprocal(stat_v[:, ts_], stat_v[:, ts_])
                nc.scalar.activation(stat_r[:, ts_], stat_v[:, ts_], AF.Sqrt)
                rse = (stat_r[:, ts_].unsqueeze(2)
                       .broadcast_to([128, Tg, h]))
                hp3 = hpre_all[:, fs].rearrange("p (t h) -> p t h", t=Tg)
                nc.vector.tensor_tensor(hp3, xc3, rse, OP.mult)
                nc.scalar.activation(hpre_all[:, fs], hpre_all[:, fs],
                                     AF.Relu)
                xout_src = hpre_all if is_skip else xnew_all
                nc.sync.dma_start(XOUTd[:, fs], xout_src[:, fs])
                if has_scores:
                    pw = (poolw_t[:].unsqueeze(1)
                          .broadcast_to([128, Tg, h]))
                    nc.vector.tensor_tensor(sq3, hp3, pw, OP.mult)
                    nc.vector.tensor_reduce(sc_all[:, ts_], sq3, AX.X,
                                            OP.add)
                    nc.scalar.activation(sc_all[:, ts_], sc_all[:, ts_],
                                         AF.Tanh, scale=wnorm)
                    nc.sync.dma_start(SCd[:, ts_], sc_all[:, ts_])
                if hn:
                    for t in range(t0, t1):
                        tp3 = psum_t.tile([128, 128], F32, tag="tp")
                        nc.tensor.transpose(tp3[:h, :],
                                            hpre_all[:, t * h:(t + 1) * h],
                                            ident_t[:])
                        hpT = nodep.tile([128, 128], F32, tag="hpT")
                        nc.scalar.copy(hpT[:h, :], tp3[:h, :])
                        hnp = psum_n.tile([128, hn], F32, tag="mlp",
                                          name="hnp")
                        nc.tensor.matmul(hnp[:], hpT[:h, :], Wn_t[:h, :])
                        nc.scalar.copy(hn_all[:, t * hn:(t + 1) * hn],
                                       hnp[:])
                    nc.sync.dma_start(
                        HNd[:, t0 * hn:t1 * hn],
                        hn_all[:, t0 * hn:t1 * hn])

            GT = 8
            for t0 in range(0, T, GT):
                finalize(t0, min(T, t0 + GT))

    nc.compile()
    try:
        from concourse.timeline_sim import TimelineSim
        nc._predicted_ns = float(TimelineSim(nc).simulate())
    except Exception:
        nc._predicted_ns = 0.0
    _PROGRAM_CACHE[key] = nc
    return nc


# --------------------------------------------------------------------------
# launch helper
# --------------------------------------------------------------------------

_IOTA = np.ascontiguousarray(
    np.broadcast_to(np.arange(128, dtype=np.float32), (128, 128)))
_IDENT = np.eye(128, dtype=np.float32)


def _bconst(t):
    v = np.array([1e-16, t * MSG_EPS, 1e-5, 0.0], np.float32)
    return np.ascontiguousarray(np.broadcast_to(v, (128, 4)))


def run_conv_launch(meta, per_core, htab_full, p, hn_W, is_skip, gated,
                    xres_full=None, poolw=None):
    """Run one conv launch across 8 cores; returns dict of gathered outputs."""
    import time
    n, Sb, T, NB, npad = (meta["n"], meta["Sb"], meta["T"], meta["NB"],
                          meta["npad"])
    h = p["Wsrc"].shape[1]
    helem = htab_full.shape[1]
    assert helem in (h, h + 1)
    hn = hn_W.shape[1] if hn_W is not None else 0
    has_scores = poolw is not None
    wnorm = float(1.0 / np.linalg.norm(poolw)) if has_scores else 0.0

    cfg = dict(h=h, helem=helem, hn=hn, T=T, NB=NB, npad=npad,
               gated=gated, is_skip=is_skip, has_scores=has_scores,
               t_scalar=float(p["t"]), wnorm=wnorm, sched=meta["sched"],
               runs=meta["runs"], has_hi=meta["has_hi"])
    t0 = time.time()
    nc = build_conv_program(cfg)
    t_compile = time.time() - t0

    HI0 = 32768
    hpad = _ceil(h, 64) * 64
    htab_pad = np.zeros((npad, hpad), np.float32)
    htab_pad[:n, :h] = htab_full[:, :h]
    wedge_b = np.ascontiguousarray(
        np.asarray(p["Wedge"], np.float32).astype(ml_dtypes.bfloat16))

    in_maps = []
    for c in range(NC):
        hown_r = np.zeros((T * 128, helem), np.float32)
        lo = c * Sb
        hi = min(n, lo + T * 128)
        if hi > lo:
            hown_r[:hi - lo] = htab_full[lo:hi]
        hown = np.ascontiguousarray(
            hown_r.reshape(T, 128, helem).transpose(1, 0, 2)
            .reshape(128, T * helem))
        m = {
            "htabL": htab_pad[:min(npad, HI0)],
            "hown": hown,
            "EB": per_core[c]["EB"],
            "EAT": per_core[c]["EAT"],
            "wedge": wedge_b,
            "W1": np.ascontiguousarray(p["W1"], np.float32),
            "W2": np.ascontiguousarray(p["W2"], np.float32),
            "iota": _IOTA,
            "ident": _IDENT,
            "bconst": _bconst(float(p["t"])),
        }
        if meta["has_hi"]:
            m["htabH"] = np.ascontiguousarray(htab_pad[HI0:])
        if hn:
            m["Wn"] = np.ascontiguousarray(hn_W, np.float32)
        if is_skip:
            xr = np.zeros((T * 128, h), np.float32)
            hi2 = min(n, lo + T * 128)
            if hi2 > lo:
                xr[:hi2 - lo] = xres_full[lo:hi2]
            m["xres"] = np.ascontiguousarray(
                xr.reshape(T, 128, h).transpose(1, 0, 2).reshape(128, T * h))
        if has_scores:
            m["poolw"] = np.ascontiguousarray(
                np.broadcast_to(poolw, (128, h)), np.float32)
        in_maps.append(m)

    t0 = time.time()
    res = run_bass_kernel_spmd(nc, in_maps, list(range(NC)))
    t_run = time.time() - t0
    LAUNCH_STATS.append({"compile_s": t_compile, "run_s": t_run,
                         "h": h, "NB": NB, "T": T,
                         "predicted_ns": getattr(nc, "_predicted_ns", 0.0)})

    def gather(name, width):
        out = np.zeros((n, width), np.float32)
        for c in range(NC):
            lo = c * Sb
            hi = min(n, lo + Sb)
            if hi > lo:
                rows = (res.results[c][name].reshape(128, T, width)
                        .transpose(1, 0, 2).reshape(T * 128, width))
                out[lo:hi] = rows[:hi - lo]
        return out

    out = {"XOUT": gather("XOUT", h)}
    if hn:
        out["HN"] = gather("HN", hn)
    if has_scores:
        sc = np.zeros(n, np.float32)
        for c in range(NC):
            lo = c * Sb
            hi = min(n, lo + Sb)
            if hi > lo:
                sc[lo:hi] = res.results[c]["SC"].T.reshape(T * 128)[:hi - lo]
        out["SC"] = sc
    return out


# --------------------------------------------------------------------------
# numpy reference fallback (also used for validation)
# --------------------------------------------------------------------------

def _np_layernorm(x, g, b):
    mu = x.mean(-1, keepdims=True)
    var = ((x - mu) ** 2).mean(-1, keepdims=True)
    return (x - mu) / np.sqrt(var + 1e-5) * g + b


def _np_genconv(x, src, dst, edge_attr, emask, p, n):
    h = x @ p["Wsrc"]
    m = np.maximum(h[src] + edge_attr @ p["Wedge"], 0.0) + MSG_EPS
    logits = np.where(emask[:, None], m * p["t"],
                      np.finfo(np.float32).min)
    mx = np.full((n, m.shape[1]), -np.inf, np.float32)
    np.maximum.at(mx, dst, logits)
    mx = np.where(np.isfinite(mx), mx, 0.0)
    ex = np.exp(logits - mx[dst]) * emask[:, None]
    den = np.zeros((n, m.shape[1]), np.float32)
    np.add.at(den, dst, ex)
    alpha = ex / (den[dst] + 1e-16)
    agg = np.zeros((n, m.shape[1]), np.float32)
    np.add.at(agg, dst, m * alpha)
    o = h + agg
    return np.maximum(o @ p["W1"] + p["b1"], 0.0) @ p["W2"] + p["b2"]


def _np_forward(x, edge_attr, edge_index, params):
    src, dst = edge_index[0].copy(), edge_index[1].copy()
    emask = np.ones(src.shape[0], bool)
    n = x.shape[0]
    x = np.asarray(x, np.float32)
    blocks = params["blocks"]
    for blk in blocks:
        x = _np_genconv(x, src, dst, edge_attr, emask, blk["conv0"], n)
        for sp in blk["skips"]:
            hpre = np.maximum(_np_layernorm(x, sp["g"], sp["b"]), 0.0)
            x = x + _np_genconv(hpre, src, dst, edge_attr, emask,
                                sp["conv"], n)
        x = np.maximum(_np_layernorm(x, blk["regu_g"], blk["regu_b"]), 0.0)
        w = np.asarray(blk["pool_w"], np.float32)
        score = np.tanh(x @ w / np.linalg.norm(w))
        k = int(np.ceil(n * POOL_RATIO))
        idx = np.argsort(-score, kind="stable")[:k]
        vals = score[idx]
        x = x[idx] * vals[:, None]
        inv = np.full(n, -1, np.int32)
        inv[idx] = np.arange(k, dtype=np.int32)
        ns, nd = inv[src], inv[dst]
        emask = emask & (ns >= 0) & (nd >= 0)
        src, dst = np.maximum(ns, 0), np.maximum(nd, 0)
        n = k
    out = x.mean(0, keepdims=True)
    for i in range(len(blocks) - 1, -1, -1):
        out = out @ np.asarray(blocks[i]["lin_W"], np.float32) + \
            np.asarray(blocks[i]["lin_b"], np.float32)
        if i != 0:
            out = np.maximum(out, 0.0)
    return out


def _params_trivial(params):
    """Check the assumptions the device programs exploit (zero biases,
    unit/zero LN params).  True for this problem's setup_inputs()."""
    try:
        for blk in params["blocks"]:
            for conv in [blk["conv0"]] + [s["conv"] for s in blk["skips"]]:
                if np.any(np.asarray(conv["b1"])) or np.any(
                        np.asarray(conv["b2"])):
                    return False
            for s in blk["skips"]:
                if (not np.allclose(np.asarray(s["g"]), 1.0)
                        or np.any(np.asarray(s["b"]))):
                    return False
            if (not np.allclose(np.asarray(blk["regu_g"]), 1.0)
                    or np.any(np.asarray(blk["regu_b"]))):
                return False
            if len(blk["skips"]) != 1:
                return False
        return True
    except Exception:
        return False


# --------------------------------------------------------------------------
# main entry
# --------------------------------------------------------------------------

def kernel(x, edge_attr, edge_index, params):
    x = np.asarray(x, np.float32)
    edge_attr = np.asarray(edge_attr, np.float32)
    ei = np.asarray(edge_index)
    in_dtype = ei.dtype
    params = _to_numpy(params)

    if not _params_trivial(params):
        return _np_forward(x, edge_attr, ei.astype(np.int64), params)

    LAUNCH_STATS.clear()
    src = ei[0].astype(np.int64)
    dst = ei[1].astype(np.int64)
    ea = edge_attr
    n = x.shape[0]
    blocks = params["blocks"]

    xcur_tab = None  # htab for the upcoming conv0 launch
    htab1 = (x @ np.asarray(blocks[0]["conv0"]["Wsrc"], np.float32)).astype(
        np.float32)
    htab_next = htab1
    gated = False

    cur_vals = None
    for bi, blk in enumerate(blocks):
        meta, per_core = pack_block_edges(src, dst, ea, n, vals=cur_vals)
        conv0 = blk["conv0"]
        skipc = blk["skips"][0]["conv"]
        # conv0 launch: outputs xnew and the skip-conv table
        r0 = run_conv_launch(meta, per_core, htab_next, conv0,
                             hn_W=np.asarray(skipc["Wsrc"], np.float32),
                             is_skip=False, gated=gated)
        xnew = r0["XOUT"]       # x after conv0
        htab_skip = r0["HN"]    # relu(LN(xnew)) @ Wsrc_skip
        # skip launch
        if bi + 1 < len(blocks):
            wn = np.asarray(blocks[bi + 1]["conv0"]["Wsrc"], np.float32)
        else:
            wn = None
        r1 = run_conv_launch(meta, per_core, htab_skip, skipc,
                             hn_W=wn, is_skip=True, gated=False,
                             xres_full=xnew,
                             poolw=np.asarray(blk["pool_w"], np.float32))
        x2b = r1["XOUT"]
        score = r1["SC"]
        # ---- host pooling ----
        k = int(np.ceil(n * POOL_RATIO))
        idx = np.argsort(-score, kind="stable")[:k]
        vals = score[idx].astype(np.float32)
        if bi + 1 < len(blocks):
            HN = r1["HN"]
            htab_next = np.concatenate(
                [HN[idx], vals[:, None]], axis=1).astype(np.float32)
            gated = True
        else:
            x_final = x2b[idx] * vals[:, None]
        inv = np.full(n, -1, np.int64)
        inv[idx] = np.arange(k)
        ns, nd = inv[src], inv[dst]
        keep = (ns >= 0) & (nd >= 0)
        src, dst, ea = ns[keep], nd[keep], ea[keep]
        n = k
        cur_vals = vals

    out = x_final.mean(0, keepdims=True).astype(np.float32)
    for i in range(len(blocks) - 1, -1, -1):
        out = out @ np.asarray(blocks[i]["lin_W"], np.float32) + \
            np.asarray(blocks[i]["lin_b"], np.float32)
        if i != 0:
            out = np.maximum(out, 0.0)
    return out.astype(np.float32)


def _to_numpy(obj):
    if isinstance(obj, dict):
        return {k: _to_numpy(v) for k, v in obj.items()}
    if isinstance(obj, (list, tuple)):
        return [_to_numpy(v) for v in obj]
    return np.asarray(obj)


# revision 36
# speedup vs baseline: 1.0196x; 1.0196x over previous
"""Trainium2 Bass kernel for nn_DyResGEN (3-block GENConv GNN with top-k pooling).

Strategy (8 NeuronCores, SPMD):
  * Edges are partitioned across cores by destination-node ranges, so the
    per-destination segment softmax/sum is core-local.  The full projected
    node table ("htab" = x @ Wsrc) is replicated per core; rows are fetched
    with indirect-DMA gathers (h[src]).
  * Per 128-edge chunk: messages m = relu(h[src] + ea0*w0 + ea1*w1) + eps and
    ex = exp(t*m) on DVE/ACT; a one-hot selection matrix S (dst-local ids vs
    iota) reduces [ex | m*ex] into per-destination-tile PSUM accumulators on
    the PE (matmul contracts the edge dim).  Softmax max-subtraction is
    dropped: logits are O(10) so exp is safe in fp32, and alpha is invariant.
  * agg = num/den with reciprocal via exp(-ln(x)); GENConv MLP per 128-node
    tile; batched LayerNorm (+relu) across the core's node slice; each launch
    also computes the next launch's projected node table slice.
  * Host does index work only: edge filtering/relabeling, top-k, packing,
    slicing/concatenation between the 6 launches (conv0+skip per block).
"""

import math
import os
import sys

import ml_dtypes
import numpy as np

try:
    import concourse  # noqa: F401
except ImportError:  # pragma: no cover
    sys.path.insert(0, "/opt/trn_rl_repo")

import concourse.bacc as bacc
import concourse.bass as bass
import concourse.mybir as mybir
import concourse.tile as tile
from concourse.bass_utils import run_bass_kernel_spmd

NC = 8           # NeuronCores
G = 8            # 128-edge chunks per batch
MSG_EPS = 1e-7
POOL_RATIO = 0.5

F32 = mybir.dt.float32
BF16 = mybir.dt.bfloat16
I32 = mybir.dt.int32
AF = mybir.ActivationFunctionType
OP = mybir.AluOpType
AX = mybir.AxisListType

LAUNCH_STATS = []  # filled per launch; read by test.py
MM_DT = F32 if os.environ.get("KMM_F32") else BF16


def _ceil(a, b):
    return -(-a // b)


# --------------------------------------------------------------------------
# host-side graph packing
# --------------------------------------------------------------------------

def pack_block_edges(src, dst, ea, n, vals=None):
    """Pack a block's edges into per-core chunked format with a schedule
    shared across cores (required for SPMD).

    Chunks are 128 edges; within a destination tile, edges whose source is
    < 32768 ("lo") come first, then "hi" sources (dma_gather idx is int16).
    Returns meta (shared schedule + per-batch gather runs) and per-core
    arrays EP [NB,128,4G] f32 (dstl|ea0|ea1|val) and IDX [NB,128,8G] i16
    (wrapped per gather run).
    """
    HI0 = 32768
    Sb = _ceil(n, NC)
    T = _ceil(Sb, 128)
    npad = (NC - 1) * Sb + T * 128
    has_hi = npad > HI0

    core = dst // Sb
    local = dst - core * Sb
    tl = local // 128
    dl = (local - tl * 128).astype(np.float32)
    hi = (src >= HI0).astype(np.int64)

    counts = np.zeros((NC, T, 2), np.int64)
    np.add.at(counts, (core, tl, hi), 1)
    K2 = -(-counts.max(axis=0) // 128)          # [T, 2] chunks per (tile, class)
    K2[:, 0] = np.maximum(K2[:, 0], (K2.sum(1) == 0).astype(np.int64))
    Ktot = int(K2.sum())
    NB = _ceil(Ktot, G)
    K2[T - 1, 0] += NB * G - Ktot               # tail pads: lo class, last tile

    # chunk stream: per tile lo chunks then hi chunks; flags per chunk
    chunk_tile = []
    chunk_hi = []
    sched = []
    for t in range(T):
        tot = int(K2[t].sum())
        k = 0
        for cls in (0, 1):
            for _ in range(int(K2[t, cls])):
                chunk_tile.append(t)
                chunk_hi.append(cls)
                sched.append((t, k == 0, k == tot - 1))
                k += 1
    assert len(sched) == NB * G
    chunk_hi = np.array(chunk_hi)

    # per-batch gather runs: maximal same-class chunk spans
    runs = []
    for b in range(NB):
        rb = []
        g = 0
        while g < G:
            g1 = g + 1
            while g1 < G and chunk_hi[b * G + g1] == chunk_hi[b * G + g]:
                g1 += 1
            rb.append((g, g1, int(chunk_hi[b * G + g])))
            g = g1
        runs.append(tuple(rb))

    # chunk slot offsets per (tile, class)
    slot0 = np.zeros((T, 2), np.int64)
    acc = 0
    for t in range(T):
        slot0[t, 0] = acc
        acc += int(K2[t, 0])
        slot0[t, 1] = acc
        acc += int(K2[t, 1])

    order = np.lexsort((hi, tl, core))
    src_s, tl_s, core_s, dl_s, hi_s = (src[order], tl[order], core[order],
                                       dl[order], hi[order])
    ea_s = ea[order]
    val_s = (vals[src[order]] if vals is not None
             else np.zeros(len(order), np.float32))

    n_chunks = NB * G
    per_core = []
    for c in range(NC):
        dstl_c = np.full((n_chunks, 128), 255.0, np.float32)
        ea0_c = np.zeros((n_chunks, 128), np.float32)
        ea1_c = np.zeros((n_chunks, 128), np.float32)
        val_c = np.zeros((n_chunks, 128), np.float32)
        idx_c = np.zeros((n_chunks, 128), np.int32)
        sel = np.where(core_s == c)[0]
        if len(sel):
            key = tl_s[sel] * 2 + hi_s[sel]
            bounds = np.searchsorted(key, np.arange(2 * T + 1))
            for t in range(T):
                for cls in (0, 1):
                    a, b = bounds[2 * t + cls], bounds[2 * t + cls + 1]
                    cnt = b - a
                    if cnt == 0:
                        continue
                    q0 = int(slot0[t, cls])
                    nch = _ceil(cnt, 128)
                    pad = nch * 128 - cnt
                    ii = sel[a:b]
                    fdl = np.concatenate(
                        [dl_s[ii], np.full(pad, 255.0, np.float32)])
                    fsr = np.concatenate(
                        [(src_s[ii] - cls * HI0).astype(np.int32),
                         np.zeros(pad, np.int32)])
                    fea = np.concatenate(
                        [ea_s[ii], np.zeros((pad, 2), np.float32)])
                    fva = np.concatenate(
                        [val_s[ii], np.zeros(pad, np.float32)])
                    dstl_c[q0:q0 + nch] = fdl.reshape(nch, 128)
                    idx_c[q0:q0 + nch] = fsr.reshape(nch, 128)
                    ea0_c[q0:q0 + nch] = fea[:, 0].reshape(nch, 128)
                    ea1_c[q0:q0 + nch] = fea[:, 1].reshape(nch, 128)
                    val_c[q0:q0 + nch] = fva.reshape(nch, 128)
        dst4 = dstl_c.reshape(NB, G, 128).transpose(0, 2, 1)
        e04 = ea0_c.reshape(NB, G, 128).transpose(0, 2, 1)
        e14 = ea1_c.reshape(NB, G, 128).transpose(0, 2, 1)
        v4 = val_c.reshape(NB, G, 128).transpose(0, 2, 1)
        EP = np.ascontiguousarray(
            np.concatenate([dst4, e04, e14, v4], axis=2), np.float32)
        # wrapped int16 idx layout, per gather run
        IDX16 = np.zeros((NB, 128, 8 * G), np.int16)
        for b in range(NB):
            for (g0, g1, cls) in runs[b]:
                ln = g1 - g0
                flat = idx_c[b * G + g0: b * G + g1].reshape(ln * 128)
                wrap = flat.reshape(8 * ln, 16).T.astype(np.int16)  # [16, 8ln]
                IDX16[b, :, 8 * g0:8 * g1] = np.tile(wrap, (8, 1))
        EAT = np.stack([ea0_c.reshape(NB, G * 128),
                        ea1_c.reshape(NB, G * 128)], axis=1)
        EB = np.concatenate([EP.view(np.uint8).reshape(NB, 128, 16 * G),
                             IDX16.view(np.uint8).reshape(NB, 128, 16 * G)],
                            axis=2)
        per_core.append({"EB": np.ascontiguousarray(EB),
                         "EAT": np.ascontiguousarray(
                             EAT.astype(ml_dtypes.bfloat16))})

    meta = {"n": n, "Sb": Sb, "T": T, "NB": NB, "npad": npad,
            "sched": tuple(sched), "runs": tuple(runs), "has_hi": has_hi}
    return meta, per_core


# --------------------------------------------------------------------------
# device program builder
# --------------------------------------------------------------------------

_PROGRAM_CACHE = {}


def build_conv_program(cfg):
    key = (cfg["h"], cfg["helem"], cfg["hn"], cfg["T"], cfg["NB"],
           cfg["npad"], cfg["gated"], cfg["is_skip"], cfg["has_scores"],
           cfg["t_scalar"], cfg["wnorm"], cfg["sched"], cfg["runs"])
    if key in _PROGRAM_CACHE:
        return _PROGRAM_CACHE[key]

    h = cfg["h"]
    helem = cfg["helem"]
    hn = cfg["hn"]
    T = cfg["T"]
    NB = cfg["NB"]
    npad = cfg["npad"]
    sched = cfg["sched"]
    gated = cfg["gated"]
    is_skip = cfg["is_skip"]
    has_scores = cfg["has_scores"]
    t_scalar = cfg["t_scalar"]
    wnorm = cfg["wnorm"]
    runs = cfg["runs"]
    has_hi = cfg["has_hi"]
    hpad = _ceil(h, 64) * 64
    HI0 = 32768
    nlo = min(npad, HI0)
    nhi = npad - nlo
    h2 = 2 * h
    nk2 = _ceil(h2, 128)

    nc = bacc.Bacc("TRN2", num_devices=NC,
                   dynamic_dma_scratch_size=65536)

    htabL = nc.dram_tensor("htabL", [nlo, hpad], F32, kind="ExternalInput")
    if has_hi:
        htabH = nc.dram_tensor("htabH", [nhi, hpad], F32,
                               kind="ExternalInput")
    hownd = nc.dram_tensor("hown", [128, T * helem], F32, kind="ExternalInput")
    EBd = nc.dram_tensor("EB", [NB, 128, 32 * G], mybir.dt.uint8,
                         kind="ExternalInput")
    EATd = nc.dram_tensor("EAT", [NB, 2, G * 128], BF16, kind="ExternalInput")
    wedged = nc.dram_tensor("wedge", [2, h], BF16, kind="ExternalInput")
    W1d = nc.dram_tensor("W1", [h, h2], F32, kind="ExternalInput")
    W2d = nc.dram_tensor("W2", [h2, h], F32, kind="ExternalInput")
    iotad = nc.dram_tensor("iota", [128, 128], F32, kind="ExternalInput")
    identd = nc.dram_tensor("ident", [128, 128], F32, kind="ExternalInput")
    bconstd = nc.dram_tensor("bconst", [128, 4], F32, kind="ExternalInput")
    if hn:
        Wnd = nc.dram_tensor("Wn", [h, hn], F32, kind="ExternalInput")
    if is_skip:
        xresd = nc.dram_tensor("xres", [128, T * h], F32, kind="ExternalInput")
    if has_scores:
        poolwd = nc.dram_tensor("poolw", [128, h], F32, kind="ExternalInput")
        SCd = nc.dram_tensor("SC", [128, T], F32, kind="ExternalOutput")
    XOUTd = nc.dram_tensor("XOUT", [128, T * h], F32, kind="ExternalOutput")
    import os as _os
    if _os.environ.get("KDBG"):
        DBGXd = nc.dram_tensor("DBGX", [128, G * h2], BF16, kind="ExternalOutput")
        DBGSd = nc.dram_tensor("DBGS", [128, G * 128], BF16, kind="ExternalOutput")
        DBGGd = nc.dram_tensor("DBGG", [128, G * hpad], F32, kind="ExternalOutput")
    if hn:
        HNd = nc.dram_tensor("HN", [128, T * hn], F32, kind="ExternalOutput")

    with tile.TileContext(nc) as tc:
        with (
            tc.tile_pool(name="const", bufs=1) as constp,
            tc.tile_pool(name="edgein", bufs=6) as edgein,
            tc.tile_pool(name="gat", bufs=6) as gatherp,
            tc.tile_pool(name="msg", bufs=4) as msgp,
            tc.tile_pool(name="exw", bufs=4) as exwp,
            tc.tile_pool(name="sel", bufs=4) as selp,
            tc.tile_pool(name="node", bufs=2) as nodep,
            tc.tile_pool(name="big", bufs=1) as bigp,
            tc.tile_pool(name="pse", bufs=2, space="PSUM") as psum_e,
            tc.tile_pool(name="psb", bufs=3, space="PSUM") as psum_b,
            tc.tile_pool(name="psn", bufs=2, space="PSUM") as psum_n,
            tc.tile_pool(name="pst", bufs=1, space="PSUM") as psum_t,
        ):
            from concourse import library_config
            nc.gpsimd.load_library(library_config.mlp)
            # ---------------- constants ----------------
            iota_t = constp.tile([128, 128], F32, tag="iota")
            nc.sync.dma_start(iota_t[:], iotad[:, :])
            iotab_t = constp.tile([128, 128], BF16, tag="iotab")
            nc.vector.tensor_copy(iotab_t[:], iota_t[:])
            ident_t = constp.tile([128, 128], F32, tag="ident")
            nc.sync.dma_start(ident_t[:], identd[:, :])
            bc_t = constp.tile([128, 4], F32, tag="bconst")
            nc.sync.dma_start(bc_t[:], bconstd[:, :])
            wedge_t = constp.tile([128, h], BF16, tag="wedge")
            nc.sync.dma_start(wedge_t[:2, :], wedged[:, :])
            W1_t = constp.tile([128, h2], F32, tag="W1")
            nc.sync.dma_start(W1_t[:h, :], W1d[:, :])
            W2_t = constp.tile([128, nk2 * h], F32, tag="W2")
            for k in range(nk2):
                kk = min(128, h2 - k * 128)
                nc.sync.dma_start(W2_t[:kk, k * h:(k + 1) * h],
                                  W2d[k * 128:k * 128 + kk, :])
            if hn:
                Wn_t = constp.tile([128, hn], F32, tag="Wn")
                nc.sync.dma_start(Wn_t[:h, :], Wnd[:, :])
            if has_scores:
                poolw_t = constp.tile([128, h], F32, tag="poolw")
                nc.sync.dma_start(poolw_t[:], poolwd[:, :])

            hown_all = bigp.tile([128, T * helem], F32, tag="hownall")
            nc.sync.dma_start(hown_all[:], hownd[:, :])
            if is_skip:
                xres_all = bigp.tile([128, T * h], F32, tag="xresall")
                nc.sync.dma_start(xres_all[:], xresd[:, :])
            xnew_all = bigp.tile([128, T * h], F32, tag="xnew")
            xc_all = bigp.tile([128, T * h], F32, tag="xc")
            hpre_all = bigp.tile([128, T * h], F32, tag="hpre")
            if hn:
                hn_all = bigp.tile([128, T * hn], F32, tag="hnall")
            if has_scores:
                sc_all = bigp.tile([128, T], F32, tag="scall")
            stat_s = bigp.tile([128, T], F32, tag="stats")
            stat_m = bigp.tile([128, T], F32, tag="statm")
            stat_v = bigp.tile([128, T], F32, tag="statv")
            stat_r = bigp.tile([128, T], F32, tag="statr")
            sq_all = bigp.tile([128, T * h], F32, tag="sqall")

            psum_of = {}

            def node_phase(t, ps):
                # agg = num/den via DVE reciprocal (avoids ACT Ln/Exp table
                # thrash against the edge-phase Exp)
                d1 = nodep.tile([128, h], F32, tag="lnd")
                nc.vector.tensor_scalar_add(d1[:], ps[:, 0:h], 1e-16)
                rden = nodep.tile([128, h], F32, tag="rden")
                nc.vector.reciprocal(rden[:], d1[:])
                agg = nodep.tile([128, h], F32, tag="agg")
                nc.vector.tensor_tensor(agg[:], ps[:, h:h2], rden[:], OP.mult)
                hown = hown_all[:, t * helem:(t + 1) * helem]
                o_t = nodep.tile([128, h], F32, tag="o")
                if gated:
                    nc.vector.scalar_tensor_tensor(
                        o_t[:], hown[:, 0:h], hown[:, h:h + 1], agg[:],
                        OP.mult, OP.add)
                else:
                    nc.vector.tensor_tensor(o_t[:], hown[:, 0:h], agg[:],
                                            OP.add)
                # MLP
                tp = psum_t.tile([128, 128], F32, tag="tp")
                nc.tensor.transpose(tp[:h, :], o_t[:], ident_t[:])
                oT = nodep.tile([128, 128], F32, tag="oT")
                nc.scalar.copy(oT[:h, :], tp[:h, :])
                y1p = psum_n.tile([128, h2], F32, tag="mlp")
                nc.tensor.matmul(y1p[:], oT[:h, :], W1_t[:h, :])
                y1r = nodep.tile([128, h2], F32, tag="y1r")
                nc.scalar.activation(y1r[:], y1p[:], AF.Relu)
                yp = psum_n.tile([128, h], F32, tag="mlp", name="yp")
                for k in range(nk2):
                    kk = min(128, h2 - k * 128)
                    tp2 = psum_t.tile([128, 128], F32, tag="tp")
                    nc.tensor.transpose(tp2[:kk, :],
                                        y1r[:, k * 128:k * 128 + kk],
                                        ident_t[:])
                    y1rT = nodep.tile([128, 128], F32, tag="y1rT")
                    nc.scalar.copy(y1rT[:kk, :], tp2[:kk, :])
                    nc.tensor.matmul(yp[:], y1rT[:kk, :],
                                     W2_t[:kk, k * h:(k + 1) * h],
                                     start=(k == 0), stop=(k == nk2 - 1))
                xn = xnew_all[:, t * h:(t + 1) * h]
                if is_skip:
                    nc.vector.tensor_tensor(
                        xn, yp[:], xres_all[:, t * h:(t + 1) * h], OP.add)
                else:
                    nc.scalar.copy(xn, yp[:])

            # ---------------- edge phase ----------------
            for b in range(NB):
                eb_t = edgein.tile([128, 32 * G], mybir.dt.uint8, tag="eb")
                nc.sync.dma_start(eb_t[:], EBd[b, :, :])
                ep_t = eb_t[:, 0:16 * G].bitcast(F32)
                ix_t = eb_t[:, 16 * G:32 * G].bitcast(mybir.dt.int16)
                gt = gatherp.tile([128, G * hpad], F32, tag="gt")
                gt3 = gt[:].rearrange("p (g e) -> p g e", g=G)
                for (g0, g1, cls) in runs[b]:
                    ln = g1 - g0
                    tab = htabH if cls else htabL
                    nc.gpsimd.dma_gather(
                        gt3[:, g0:g1, :], tab[:, :],
                        ix_t[:, 8 * g0:8 * g1], ln * 128, ln * 128, hpad)
                # eemb = ea^T @ Wedge on the PE (per-chunk stationary)
                eat_t = edgein.tile([128, G * 128], BF16, tag="eat")
                nc.sync.dma_start(eat_t[:2, :], EATd[b, :, :])
                NH = 1 if G * h * 4 <= 2048 else 2
                G2 = G // NH
                emb_halves = []
                for hh in range(NH):
                    emb_ps = psum_b.tile([128, G2 * h], F32, tag="emb",
                                         name="emb")
                    for gg in range(G2):
                        g = hh * G2 + gg
                        nc.tensor.matmul(emb_ps[:, gg * h:(gg + 1) * h],
                                         eat_t[:2, g * 128:(g + 1) * 128],
                                         wedge_t[:2, :], start=True,
                                         stop=True)
                    emb_halves.append(emb_ps)
                d_t = msgp.tile([128, G * h], F32, tag="d")
                d3 = d_t[:].rearrange("p (g h) -> p g h", g=G)
                if gated:
                    for g in range(G):
                        emb_ps = emb_halves[g // G2]
                        gg = g % G2
                        nc.vector.scalar_tensor_tensor(
                            d_t[:, g * h:(g + 1) * h], gt3[:, g, 0:h],
                            ep_t[:, 3 * G + g:3 * G + g + 1],
                            emb_ps[:, gg * h:(gg + 1) * h],
                            OP.mult, OP.add)
                else:
                    for hh in range(NH):
                        nc.vector.tensor_tensor(
                            d3[:, hh * G2:(hh + 1) * G2, :],
                            emb_halves[hh][:].rearrange(
                                "p (g h) -> p g h", g=G2),
                            gt3[:, hh * G2:(hh + 1) * G2, 0:h], OP.add)
                # me = relu(d) + eps on DVE (tensor_scalar, two scalars)
                m_t = msgp.tile([128, G * h], BF16, tag="m")
                nc.vector.tensor_scalar(m_t[:], d_t[:], 0.0, MSG_EPS,
                                        OP.max, OP.add)
                exw = exwp.tile([128, G * h2], MM_DT, tag="exw")
                exv = exw[:].rearrange("p (g h) -> p g h", g=G)
                mv = m_t[:].rearrange("p (g h) -> p g h", g=G)
                nc.scalar.activation(exv[:, :, 0:h], mv, AF.Exp,
                                     scale=t_scalar)
                nc.vector.tensor_tensor(exv[:, :, h:h2], mv,
                                        exv[:, :, 0:h], OP.mult)
                # S one-hot: per-chunk tensor_scalar vs bf16 iota (4x mode)
                S_t = selp.tile([128, G * 128], MM_DT, tag="S")
                Sv = S_t[:].rearrange("p (g s) -> p g s", g=G)
                for g in range(G):
                    nc.vector.tensor_scalar(
                        S_t[:, g * 128:(g + 1) * 128], iotab_t[:],
                        ep_t[:, g:g + 1], None, OP.is_equal)
                if _os.environ.get("KDBG") and b == 0:
                    nc.sync.dma_start(DBGXd[:, :], exw[:])
                    nc.sync.dma_start(DBGSd[:, :], S_t[:])
                    nc.sync.dma_start(DBGGd[:, :], gt[:])
                for g in range(G):
                    t_id, st, sp = sched[b * G + g]
                    if st:
                        psum_of[t_id] = psum_e.tile([128, h2], F32, tag="eps", name="eps")
                    ps = psum_of[t_id]
                    nc.tensor.matmul(ps[:], Sv[:, g, :], exv[:, g, :],
                                     start=st, stop=sp)
                    if sp:
                        node_phase(t_id, ps)

            # ---- batched LayerNorm (+relu), scores, next-table projection,
            # emitted per tile-group with disjoint AP ranges so the scheduler
            # can overlap finalization with the remaining edge phase ----
            def finalize(t0, t1):
                Tg = t1 - t0
                fs = slice(t0 * h, t1 * h)
                ts_ = slice(t0, t1)
                xa3 = (xnew_all[:, fs].rearrange("p (t h) -> p t h", t=Tg))
                nc.vector.tensor_reduce(stat_s[:, ts_], xa3, AX.X, OP.add)
                nc.scalar.activation(stat_m[:, ts_], stat_s[:, ts_], AF.Copy,
                                     scale=-1.0 / h)
                mue = (stat_m[:, ts_].unsqueeze(2)
                       .broadcast_to([128, Tg, h]))
                xc3 = xc_all[:, fs].rearrange("p (t h) -> p t h", t=Tg)
                nc.vector.tensor_tensor(xc3, xa3, mue, OP.add)
                nc.scalar.activation(sq_all[:, fs], xc_all[:, fs], AF.Square)
                sq3 = sq_all[:, fs].rearrange("p (t h) -> p t h", t=Tg)
                nc.vector.tensor_reduce(stat_v[:, ts_], sq3, AX.X, OP.add)
                nc.scalar.activation(stat_v[:, ts_], stat_v[:, ts_], AF.Copy,
                                     scale=1.0 / h, bias=1e-5)
                nc.vector.reciprocal(stat_v[:, ts_], stat_v[:, ts_])
                nc.scalar.activation(stat_r[:, ts_], stat_v[:, ts_], AF.Sqrt)
                rse = (stat_r[:, ts_].unsqueeze(2)
                       .broadcast_to([128, Tg, h]))
                hp3 = hpre_all[:, fs].rearrange("p (t h) -> p t h", t=Tg)
                nc.vector.tensor_tensor(hp3, xc3, rse, OP.mult)
                nc.scalar.activation(hpre_all[:, fs], hpre_all[:, fs],
                                     AF.Relu)
                xout_src = hpre_all if is_skip else xnew_all
                nc.sync.dma_start(XOUTd[:, fs], xout_src[:, fs])
                if has_scores:
                    pw = (poolw_t[:].unsqueeze(1)
                          .broadcast_to([128, Tg, h]))
                    nc.vector.tensor_tensor(sq3, hp3, pw, OP.mult)
                    nc.vector.tensor_reduce(sc_all[:, ts_], sq3, AX.X,
                                            OP.add)
                    nc.scalar.activation(sc_all[:, ts_], sc_all[:, ts_],
                                         AF.Tanh, scale=wnorm)
                    nc.sync.dma_start(SCd[:, ts_], sc_all[:, ts_])
                if hn:
                    for t in range(t0, t1):
                        tp3 = psum_t.tile([128, 128], F32, tag="tp")
                        nc.tensor.transpose(tp3[:h, :],
                                            hpre_all[:, t * h:(t + 1) * h],
                                            ident_t[:])
                        hpT = nodep.tile([128, 128], F32, tag="hpT")
                        nc.scalar.copy(hpT[:h, :], tp3[:h, :])
                        hnp = psum_n.tile([128, hn], F32, tag="mlp",
                                          name="hnp")
                        nc.tensor.matmul(hnp[:], hpT[:h, :], Wn_t[:h, :])
                        nc.scalar.copy(hn_all[:, t * hn:(t + 1) * hn],
                                       hnp[:])
                    nc.sync.dma_start(
                        HNd[:, t0 * hn:t1 * hn],
                        hn_all[:, t0 * hn:t1 * hn])

            GT = 8
            for t0 in range(0, T, GT):
                finalize(t0, min(T, t0 + GT))

    nc.compile()
    try:
        from concourse.timeline_sim import TimelineSim
        nc._predicted_ns = float(TimelineSim(nc).simulate())
    except Exception:
        nc._predicted_ns = 0.0
    _PROGRAM_CACHE[key] = nc
    return nc


# --------------------------------------------------------------------------
# launch helper
# --------------------------------------------------------------------------

_IOTA = np.ascontiguousarray(
    np.broadcast_to(np.arange(128, dtype=np.float32), (128, 128)))
_IDENT = np.eye(128, dtype=np.float32)


def _bconst(t):
    v = np.array([1e-16, t * MSG_EPS, 1e-5, 0.0], np.float32)
    return np.ascontiguousarray(np.broadcast_to(v, (128, 4)))


def run_conv_launch(meta, per_core, htab_full, p, hn_W, is_skip, gated,
                    xres_full=None, poolw=None):
    """Run one conv launch across 8 cores; returns dict of gathered outputs."""
    import time
    n, Sb, T, NB, npad = (meta["n"], meta["Sb"], meta["T"], meta["NB"],
                          meta["npad"])
    h = p["Wsrc"].shape[1]
    helem = htab_full.shape[1]
    assert helem in (h, h + 1)
    hn = hn_W.shape[1] if hn_W is not None else 0
    has_scores = poolw is not None
    wnorm = float(1.0 / np.linalg.norm(poolw)) if has_scores else 0.0

    cfg = dict(h=h, helem=helem, hn=hn, T=T, NB=NB, npad=npad,
               gated=gated, is_skip=is_skip, has_scores=has_scores,
               t_scalar=float(p["t"]), wnorm=wnorm, sched=meta["sched"],
               runs=meta["runs"], has_hi=meta["has_hi"])
    t0 = time.time()
    nc = build_conv_program(cfg)
    t_compile = time.time() - t0

    HI0 = 32768
    hpad = _ceil(h, 64) * 64
    htab_pad = np.zeros((npad, hpad), np.float32)
    htab_pad[:n, :h] = htab_full[:, :h]
    wedge_b = np.ascontiguousarray(
        np.asarray(p["Wedge"], np.float32).astype(ml_dtypes.bfloat16))

    in_maps = []
    for c in range(NC):
        hown_r = np.zeros((T * 128, helem), np.float32)
        lo = c * Sb
        hi = min(n, lo + T * 128)
        if hi > lo:
            hown_r[:hi - lo] = htab_full[lo:hi]
        hown = np.ascontiguousarray(
            hown_r.reshape(T, 128, helem).transpose(1, 0, 2)
            .reshape(128, T * helem))
        m = {
            "htabL": htab_pad[:min(npad, HI0)],
            "hown": hown,
            "EB": per_core[c]["EB"],
            "EAT": per_core[c]["EAT"],
            "wedge": wedge_b,
            "W1": np.ascontiguousarray(p["W1"], np.float32),
            "W2": np.ascontiguousarray(p["W2"], np.float32),
            "iota": _IOTA,
            "ident": _IDENT,
            "bconst": _bconst(float(p["t"])),
        }
        if meta["has_hi"]:
            m["htabH"] = np.ascontiguousarray(htab_pad[HI0:])
        if hn:
            m["Wn"] = np.ascontiguousarray(hn_W, np.float32)
        if is_skip:
            xr = np.zeros((T * 128, h), np.float32)
            hi2 = min(n, lo + T * 128)
            if hi2 > lo:
                xr[:hi2 - lo] = xres_full[lo:hi2]
            m["xres"] = np.ascontiguousarray(
                xr.reshape(T, 128, h).transpose(1, 0, 2).reshape(128, T * h))
        if has_scores:
            m["poolw"] = np.ascontiguousarray(
                np.broadcast_to(poolw, (128, h)), np.float32)
        in_maps.append(m)

    t0 = time.time()
    res = run_bass_kernel_spmd(nc, in_maps, list(range(NC)))
    t_run = time.time() - t0
    LAUNCH_STATS.append({"compile_s": t_compile, "run_s": t_run,
                         "h": h, "NB": NB, "T": T,
                         "predicted_ns": getattr(nc, "_predicted_ns", 0.0)})

    def gather(name, width):
        out = np.zeros((n, width), np.float32)
        for c in range(NC):
            lo = c * Sb
            hi = min(n, lo + Sb)
            if hi > lo:
                rows = (res.results[c][name].reshape(128, T, width)
                        .transpose(1, 0, 2).reshape(T * 128, width))
                out[lo:hi] = rows[:hi - lo]
        return out

    out = {"XOUT": gather("XOUT", h)}
    if hn:
        out["HN"] = gather("HN", hn)
    if has_scores:
        sc = np.zeros(n, np.float32)
        for c in range(NC):
            lo = c * Sb
            hi = min(n, lo + Sb)
            if hi > lo:
                sc[lo:hi] = res.results[c]["SC"].T.reshape(T * 128)[:hi - lo]
        out["SC"] = sc
    return out


# --------------------------------------------------------------------------
# numpy reference fallback (also used for validation)
# --------------------------------------------------------------------------

def _np_layernorm(x, g, b):
    mu = x.mean(-1, keepdims=True)
    var = ((x - mu) ** 2).mean(-1, keepdims=True)
    return (x - mu) / np.sqrt(var + 1e-5) * g + b


def _np_genconv(x, src, dst, edge_attr, emask, p, n):
    h = x @ p["Wsrc"]
    m = np.maximum(h[src] + edge_attr @ p["Wedge"], 0.0) + MSG_EPS
    logits = np.where(emask[:, None], m * p["t"],
                      np.finfo(np.float32).min)
    mx = np.full((n, m.shape[1]), -np.inf, np.float32)
    np.maximum.at(mx, dst, logits)
    mx = np.where(np.isfinite(mx), mx, 0.0)
    ex = np.exp(logits - mx[dst]) * emask[:, None]
    den = np.zeros((n, m.shape[1]), np.float32)
    np.add.at(den, dst, ex)
    alpha = ex / (den[dst] + 1e-16)
    agg = np.zeros((n, m.shape[1]), np.float32)
    np.add.at(agg, dst, m * alpha)
    o = h + agg
    return np.maximum(o @ p["W1"] + p["b1"], 0.0) @ p["W2"] + p["b2"]


def _np_forward(x, edge_attr, edge_index, params):
    src, dst = edge_index[0].copy(), edge_index[1].copy()
    emask = np.ones(src.shape[0], bool)
    n = x.shape[0]
    x = np.asarray(x, np.float32)
    blocks = params["blocks"]
    for blk in blocks:
        x = _np_genconv(x, src, dst, edge_attr, emask, blk["conv0"], n)
        for sp in blk["skips"]:
            hpre = np.maximum(_np_layernorm(x, sp["g"], sp["b"]), 0.0)
            x = x + _np_genconv(hpre, src, dst, edge_attr, emask,
                                sp["conv"], n)
        x = np.maximum(_np_layernorm(x, blk["regu_g"], blk["regu_b"]), 0.0)
        w = np.asarray(blk["pool_w"], np.float32)
        score = np.tanh(x @ w / np.linalg.norm(w))
        k = int(np.ceil(n * POOL_RATIO))
        idx = np.argsort(-score, kind="stable")[:k]
        vals = score[idx]
        x = x[idx] * vals[:, None]
        inv = np.full(n, -1, np.int32)
        inv[idx] = np.arange(k, dtype=np.int32)
        ns, nd = inv[src], inv[dst]
        emask = emask & (ns >= 0) & (nd >= 0)
        src, dst = np.maximum(ns, 0), np.maximum(nd, 0)
        n = k
    out = x.mean(0, keepdims=True)
    for i in range(len(blocks) - 1, -1, -1):
        out = out @ np.asarray(blocks[i]["lin_W"], np.float32) + \
            np.asarray(blocks[i]["lin_b"], np.float32)
        if i != 0:
            out = np.maximum(out, 0.0)
    return out


def _params_trivial(params):
    """Check the assumptions the device programs exploit (zero biases,
    unit/zero LN params).  True for this problem's setup_inputs()."""
    try:
        for blk in params["blocks"]:
            for conv in [blk["conv0"]] + [s["conv"] for s in blk["skips"]]:
                if np.any(np.asarray(conv["b1"])) or np.any(
                        np.asarray(conv["b2"])):
                    return False
            for s in blk["skips"]:
                if (not np.allclose(np.asarray(s["g"]), 1.0)
                        or np.any(np.asarray(s["b"]))):
                    return False
            if (not np.allclose(np.asarray(blk["regu_g"]), 1.0)
                    or np.any(np.asarray(blk["regu_b"]))):
                return False
            if len(blk["skips"]) != 1:
                return False
        return True
    except Exception:
        return False


# --------------------------------------------------------------------------
# main entry
# --------------------------------------------------------------------------

def kernel(x, edge_attr, edge_index, params):
    x = np.asarray(x, np.float32)
    edge_attr = np.asarray(edge_attr, np.float32)
    ei = np.asarray(edge_index)
    in_dtype = ei.dtype
    params = _to_numpy(params)

    if not _params_trivial(params):
        return _np_forward(x, edge_attr, ei.astype(np.int64), params)

    LAUNCH_STATS.clear()
    src = ei[0].astype(np.int64)
    dst = ei[1].astype(np.int64)
    ea = edge_attr
    n = x.shape[0]
    blocks = params["blocks"]

    xcur_tab = None  # htab for the upcoming conv0 launch
    htab1 = (x @ np.asarray(blocks[0]["conv0"]["Wsrc"], np.float32)).astype(
        np.float32)
    htab_next = htab1
    gated = False

    cur_vals = None
    for bi, blk in enumerate(blocks):
        meta, per_core = pack_block_edges(src, dst, ea, n, vals=cur_vals)
        conv0 = blk["conv0"]
        skipc = blk["skips"][0]["conv"]
        # conv0 launch: outputs xnew and the skip-conv table
        r0 = run_conv_launch(meta, per_core, htab_next, conv0,
                             hn_W=np.asarray(skipc["Wsrc"], np.float32),
                             is_skip=False, gated=gated)
        xnew = r0["XOUT"]       # x after conv0
        htab_skip = r0["HN"]    # relu(LN(xnew)) @ Wsrc_skip
        # skip launch
        if bi + 1 < len(blocks):
            wn = np.asarray(blocks[bi + 1]["conv0"]["Wsrc"], np.float32)
        else:
            wn = None
        r1 = run_conv_launch(meta, per_core, htab_skip, skipc,
                             hn_W=wn, is_skip=True, gated=False,
                             xres_full=xnew,
                             poolw=np.asarray(blk["pool_w"], np.float32))
        x2b = r1["XOUT"]
        score = r1["SC"]
        # ---- host pooling ----
        k = int(np.ceil(n * POOL_RATIO))
        idx = np.argsort(-score, kind="stable")[:k]
        vals = score[idx].astype(np.float32)
        if bi + 1 < len(blocks):
            HN = r1["HN"]
            htab_next = np.concatenate(
                [HN[idx], vals[:, None]], axis=1).astype(np.float32)
            gated = True
        else:
            x_final = x2b[idx] * vals[:, None]
        inv = np.full(n, -1, np.int64)
        inv[idx] = np.arange(k)
        ns, nd = inv[src], inv[dst]
        keep = (ns >= 0) & (nd >= 0)
        src, dst, ea = ns[keep], nd[keep], ea[keep]
        n = k
        cur_vals = vals

    out = x_final.mean(0, keepdims=True).astype(np.float32)
    for i in range(len(blocks) - 1, -1, -1):
        out = out @ np.asarray(blocks[i]["lin_W"], np.float32) + \
            np.asarray(blocks[i]["lin_b"], np.float32)
        if i != 0:
            out = np.maximum(out, 0.0)
    return out.astype(np.float32)


def _to_numpy(obj):
    if isinstance(obj, dict):
        return {k: _to_numpy(v) for k, v in obj.items()}
    if isinstance(obj, (list, tuple)):
        return [_to_numpy(v) for v in obj]
    return np.asarray(obj)
